# revision 14
# baseline (speedup 1.0000x reference)
"""Trainium2 Bass kernel for a 2-layer GCN encoder (adversarial GCN, N=10000).

Math (per reference):
  conv(X, W, b) = Dinv (A + I) Dinv X W + b,  Dinv = diag(deg^-1/2),
  deg = in-degree(dst) + 1,  A[d, s] = multiplicity of edge (s -> d).
  out = conv2(conv1(x) + perturb_first) + perturb_last

Strategy (8 NeuronCores, 1D node partitioning by dst):
  Let B' = Dinv_src-scaled count matrix: B'[d,s] = (A+I)[d,s] * dinv_s,
  built on host from the edge list + degree histogram (structure data) and
  stored fp8e4m3.  Each core owns 1250 dst rows; its B'^T shard
  [10000, 1250] is loaded ONCE into SBUF as 39 resident [128, 2, 1250]
  k-pair tiles (+ a 16-row tail) and reused by both layers.  The two big
  B-matmuls run in DoubleRow perf mode (fp8 x fp8, 2 contraction rows per
  PE pass) against fp8 features, contracting the narrow 256-col feature
  dim:
    t3 = dinv_d * (B' @ x8)            (layer-1 aggregate, f16 [feat, node])
    s2 = t3^T @ (W1@W2) + (P1+b1) @ W2 (fused W1/W2 stage: W12 is computed
                                        once on device; the perturbation
                                        rides as extra matmul weights, so
                                        s1 is never materialized)
    u  = B' @ fp8(s2);  out = dinv_d * u + P2 + b2
  s2 is exchanged through a single fp8 AllGather (0.31 MB/rank) into one
  shared [10000, 256] buffer; layer 2 gathers 256-row chunks from it with
  two DMAs per chunk.  All intermediates stay in "transposed" layout
  [feat, node] except s2, which is produced naturally row-major for the
  exchange.

Host does index/structure preprocessing (degree histogram, dinv-folded
B'^T shard construction in the DoubleRow tile layout, row-shard slicing /
transposition of perturbs) plus dtype down-casts of the dense input
streams (x -> fp8e4m3, perturbs/weights -> f16); all arithmetic on tensor
data runs on device.
"""

import sys

sys.path.insert(0, "/opt/trn_rl_repo")

import numpy as np
import ml_dtypes

import concourse.bass as bass
import concourse.tile as tile
from concourse import bacc, mybir
from concourse.bass_utils import run_bass_kernel_spmd

N_CORES = 8
N = 10000
R = N // N_CORES  # 1250 rows per core
F_IN = 256
F_HID = 512
F_OUT = 256
PB = 1280  # padded per-core row block (10 full m-tiles)
NPAD = N_CORES * PB  # 10240 = 40 * 256: uniform DoubleRow chunks, no tail
KC = 40  # 256-row DoubleRow contraction chunks

# dst columns per core split into PSUM-bank-sized chunks (<=512 fp32)
N_CHUNKS = [(0, 512), (512, 512), (1024, 226)]
# 1250 = 9*128 + 98 row tiles for the fused W12 (natural-layout) matmul
M_TILES = [(m * 128, min(128, R - m * 128)) for m in range((R + 127) // 128)]
# p1d column-chunk j covering m-tile m0
J_OF_MTILE = [next(j for j, (n0, nw) in enumerate(N_CHUNKS)
                   if n0 <= m0 < n0 + nw) for (m0, _) in M_TILES]

F8 = mybir.dt.float8e4
F8_NP = ml_dtypes.float8_e4m3
F16 = mybir.dt.float16
F32 = mybir.dt.float32
DR = mybir.MatmulPerfMode.DoubleRow
ADD = mybir.AluOpType.add
MUL = mybir.AluOpType.mult


def build_nc(repeat: int = 1, skip: frozenset = frozenset()):
    """skip: subset of {"L1", "D", "AG", "L2"} — timing-attribution
    variants (outputs are garbage when any phase is skipped)."""
    nc = bacc.Bacc("TRN2", target_bir_lowering=False, debug=False, num_devices=N_CORES)

    # ---- DRAM I/O -------------------------------------------------------
    # bt/x pre-laid out on host as [k, p, t, cols]: chunk k is one DMA
    x_d = nc.dram_tensor("x8", [KC, 128, 2, F_IN], F8, kind="ExternalInput")
    bt_d = nc.dram_tensor("bt", [KC, 128, 2, R], F8, kind="ExternalInput")
    p1t_d = nc.dram_tensor("p1t", [F_HID, R], F16, kind="ExternalInput")
    p2t_d = nc.dram_tensor("p2t", [F_OUT, R], F16, kind="ExternalInput")
    dinvloc_d = nc.dram_tensor("dinvloc", [R], F32, kind="ExternalInput")
    w1t_d = nc.dram_tensor("w1t", [F_HID, F_IN], F16, kind="ExternalInput")  # W1^T
    w2_d = nc.dram_tensor("w2", [F_HID, F_OUT], F16, kind="ExternalInput")
    b1_d = nc.dram_tensor("b1", [F_HID], F32, kind="ExternalInput")
    b2_d = nc.dram_tensor("b2", [F_OUT], F32, kind="ExternalInput")
    out_d = nc.dram_tensor("outT", [F_OUT, R], F32, kind="ExternalOutput")

    with tile.TileContext(nc) as tc:
        with (
            tc.tile_pool(name="const", bufs=1) as cpool,
            tc.tile_pool(name="btr", bufs=1) as btrp,
            tc.tile_pool(name="ps", bufs=8, space="PSUM") as ps,
            tc.tile_pool(name="xio", bufs=6) as xio,
            tc.tile_pool(name="s2f", bufs=6) as s2fp,
            tc.tile_pool(name="t3", bufs=1) as t3p,
            tc.tile_pool(name="p1d", bufs=1) as p1dp,
            tc.tile_pool(name="pio", bufs=4) as pio,
            tc.tile_pool(name="tmp", bufs=4) as tmpp,
            tc.tile_pool(name="dram", bufs=1, space="DRAM") as dram,
        ):
            def load_chunk(k, it):
                """One DMA each for the resident bt and x tiles of chunk k."""
                bt = btrp.tile([128, 2, R], F8, name=f"btr{k}_{it}", tag=f"btr{k}")
                (nc.sync if k % 2 else nc.scalar).dma_start(bt[:], bt_d[k])
                xf = None
                if "L1" not in skip:
                    xf = xio.tile([128, 2, F_IN], F8, tag="xio")
                    (nc.scalar if k % 2 else nc.sync).dma_start(xf[:], x_d[k])
                return bt, xf

            # iteration-0 prefetch ahead of the descriptor-heavy constant
            # loads below — the first matmul chain needs only these
            pf0 = {k: load_chunk(k, 0) for k in range(6)}

            # ---- constants ---------------------------------------------
            dinv_row = cpool.tile([128, R], F32)
            nc.sync.dma_start(
                dinv_row[:], dinvloc_d.ap().unsqueeze(0).broadcast_to((128, R))
            )
            b1t = []
            for m in range(4):
                t = cpool.tile([128, 1], F32, name=f"b1t{m}")
                nc.sync.dma_start(t[:], b1_d[m * 128:(m + 1) * 128].unsqueeze(1))
                b1t.append(t)
            b2t = []
            for m in range(2):
                t = cpool.tile([128, 1], F32, name=f"b2t{m}")
                nc.sync.dma_start(t[:], b2_d[m * 128:(m + 1) * 128].unsqueeze(1))
                b2t.append(t)
            w2h = []
            for kk in range(4):
                wh = cpool.tile([128, F_OUT], F16, name=f"w2h{kk}")
                nc.sync.dma_start(wh[:], w2_d[kk * 128:(kk + 1) * 128, :])
                w2h.append(wh)
            w1tt = []
            for kk in range(4):
                wh = cpool.tile([128, F_IN], F16, name=f"w1tt{kk}")
                nc.sync.dma_start(wh[:], w1t_d[kk * 128:(kk + 1) * 128, :])
                w1tt.append(wh)
            # W12 = W1 @ W2 on device, once: [256, 256] f16 as 2 row-tiles
            w12h = []
            for f in range(2):
                psw = ps.tile([128, 512], F32, name=f"psw{f}", tag="ps")
                for kk in range(4):
                    nc.tensor.matmul(
                        psw[:, :F_OUT],
                        w1tt[kk][:, f * 128:(f + 1) * 128],
                        w2h[kk][:],
                        start=(kk == 0),
                        stop=(kk == 3),
                    )
                wh = cpool.tile([128, F_OUT], F16, name=f"w12h{f}")
                nc.vector.tensor_copy(wh[:], psw[:, :F_OUT])
                w12h.append(wh)
            ztc = cpool.tile([128, F_OUT], F8, name="ztc")
            nc.gpsimd.memset(ztc[:], 0.0)

            for it in range(repeat):
                # collective bounce buffers (Shared DRAM output)
                # DR chunk layout [lq, p, t, f]: local row lq*256 + t*128 + p.
                # Split exchange: part A = lq 0..2, part B = lq 3..4 (incl pad);
                # chunk k = rank*5 + lq lives at ccoA[rank*3+lq] / ccoB[rank*2+lq-3]
                cc_in = dram.tile([PB // 256, 128, 2, F_OUT], F8,
                                  name=f"cc_in{it}", tag=f"cci{it}")
                cc_outA = dram.tile([N_CORES * 3, 128, 2, F_OUT], F8,
                                    addr_space="Shared",
                                    name=f"cc_outA{it}", tag=f"ccoA{it}")
                cc_outB = dram.tile([N_CORES * 2, 128, 2, F_OUT], F8,
                                    addr_space="Shared",
                                    name=f"cc_outB{it}", tag=f"ccoB{it}")

                pf = pf0 if it == 0 else {k: load_chunk(k, it) for k in range(3)}
                btr = {}  # resident bt tile per chunk, reused by layer 2

                # ============ Layer 1: t3 = dinv_d * (B' @ x8)^T ============
                t3T = [t3p.tile([128, R], F16, name=f"t3T{f}_{it}", tag=f"t3T{f}")
                       for f in range(2)]
                ps1 = [
                    [ps.tile([128, 512], F32, name=f"ps1_{f}_{j}_{it}", tag="ps")
                     for j in range(3)]
                    for f in range(2)
                ]
                for k in range(KC):
                    bt, xf = pf[k] if k in pf else load_chunk(k, it)
                    btr[k] = bt
                    if "L1" in skip:
                        continue
                    for f in range(2):
                        lhsT = xf[:, :, f * 128:(f + 1) * 128]
                        for j, (n0, nw) in enumerate(N_CHUNKS):
                            nc.tensor.matmul(
                                ps1[f][j][:, :nw],
                                lhsT,
                                bt[:, :, n0:n0 + nw],
                                start=(k == 0),
                                stop=(k == KC - 1),
                                perf_mode=DR,
                            )
                    if k == 12:
                        # p1d[hid][j] = P1^T + b1 (f16) — lands mid-L1-stream
                        # so it is ready for the fused W12 stage right after
                        p1d = [[None] * 3 for _ in range(4)]
                        for hid in range(4):
                            for j, (n0, nw) in enumerate(N_CHUNKS):
                                p1f = pio.tile([128, 512], F16, tag="pio")
                                nc.sync.dma_start(
                                    p1f[:, :nw],
                                    p1t_d[hid * 128:(hid + 1) * 128, n0:n0 + nw],
                                )
                                pd = p1dp.tile([128, 512], F16,
                                               name=f"p1d{hid}_{j}_{it}",
                                               tag=f"p1d{hid}{j}")
                                nc.vector.tensor_scalar_add(
                                    pd[:, :nw], p1f[:, :nw], b1t[hid][:]
                                )
                                p1d[hid][j] = pd
                if "L1" not in skip:
                    for f in range(2):
                        for j, (n0, nw) in enumerate(N_CHUNKS):
                            nc.vector.tensor_mul(
                                t3T[f][:, n0:n0 + nw],
                                ps1[f][j][:, :nw],
                                dinv_row[:, n0:n0 + nw],
                            )
                else:
                    for f in range(2):
                        nc.gpsimd.memset(t3T[f][:], 0.0)
                    p1d = [[None] * 3 for _ in range(4)]
                    for hid in range(4):
                        for j in range(3):
                            pd = p1dp.tile([128, 512], F16,
                                           name=f"p1d{hid}_{j}_{it}",
                                           tag=f"p1d{hid}{j}")
                            nc.gpsimd.memset(pd[:], 0.0)
                            p1d[hid][j] = pd

                # ==== fused W12 stage: s2 = t3^T @ W12 + p1d^T @ W2 (fp8) ====
                if "D" in skip:
                    zt = tmpp.tile([128, F_OUT], F8, tag="s2h")
                    nc.gpsimd.memset(zt[:], 0.0)
                    for (m0, mw) in M_TILES:
                        nc.sync.dma_start(
                            cc_in[m0 // 256, :mw, (m0 // 128) % 2, :], zt[:mw, :])
                    if "AG" not in skip and "AGS" not in skip:
                        nc.gpsimd.collective_compute(
                            "AllGather", mybir.AluOpType.bypass,
                            replica_groups=[list(range(N_CORES))],
                            ins=[cc_in[0:3].opt()], outs=[cc_outA.opt()],
                        )
                else:
                    for mi, (m0, mw) in enumerate(M_TILES):
                        if mi == 6 and "AG" not in skip and "AGS" not in skip:
                            nc.gpsimd.collective_compute(
                                "AllGather", mybir.AluOpType.bypass,
                                replica_groups=[list(range(N_CORES))],
                                ins=[cc_in[0:3].opt()], outs=[cc_outA.opt()],
                            )
                        j = J_OF_MTILE[mi]
                        n0 = N_CHUNKS[j][0]
                        psd = ps.tile([128, 512], F32, name=f"psd_{m0}_{it}",
                                      tag="ps")
                        for kk in range(4):
                            nc.tensor.matmul(
                                psd[:mw, :F_OUT],
                                p1d[kk][j][:, m0 - n0:m0 - n0 + mw],
                                w2h[kk][:],
                                start=(kk == 0),
                                stop=False,
                            )
                        for kk in range(2):
                            nc.tensor.matmul(
                                psd[:mw, :F_OUT],
                                t3T[kk][:, m0:m0 + mw],
                                w12h[kk][:],
                                start=False,
                                stop=(kk == 1),
                            )
                        s2h = tmpp.tile([128, F_OUT], F8, tag="s2h")
                        nc.vector.tensor_copy(s2h[:mw, :], psd[:mw, :F_OUT])
                        nc.sync.dma_start(
                            cc_in[m0 // 256, :mw, (m0 // 128) % 2, :], s2h[:mw, :])
                nc.sync.dma_start(cc_in[4, R - 1152:128, 1, :], ztc[:PB - R, :])

                if "AGS" in skip:
                    # small-payload AllGather probe
                    nc.gpsimd.collective_compute(
                        "AllGather", mybir.AluOpType.bypass,
                        replica_groups=[list(range(N_CORES))],
                        ins=[cc_in[0:1].opt()], outs=[cc_outA[0:8].opt()],
                    )
                elif "AG" not in skip:
                    nc.gpsimd.collective_compute(
                        "AllGather", mybir.AluOpType.bypass,
                        replica_groups=[list(range(N_CORES))],
                        ins=[cc_in[3:5].opt()], outs=[cc_outB.opt()],
                    )
                else:
                    nc.sync.dma_start(cc_outA[0:3], cc_in[0:3])
                    nc.sync.dma_start(cc_outB[0:2], cc_in[3:5])

                # ============ Layer 2: uT = (B' @ s2_full)^T ================
                if "L2" in skip:
                    continue
                # p2d = P2^T + b2 (f16, overlaps the AllGather)
                p2d = [[None] * 3 for _ in range(2)]
                for f in range(2):
                    for j, (n0, nw) in enumerate(N_CHUNKS):
                        p2f = pio.tile([128, 512], F16, tag="pio")
                        nc.sync.dma_start(
                            p2f[:, :nw], p2t_d[f * 128:(f + 1) * 128, n0:n0 + nw]
                        )
                        pd = tmpp.tile([128, 512], F16, name=f"p2d{f}_{j}_{it}",
                                       tag=f"p2d{f}{j}")
                        nc.vector.tensor_scalar_add(pd[:, :nw], p2f[:, :nw], b2t[f][:])
                        p2d[f][j] = pd
                ps2 = [
                    [ps.tile([128, 512], F32, name=f"ps2_{f}_{j}_{it}", tag="ps")
                     for j in range(3)]
                    for f in range(2)
                ]
                k_order = ([c * 5 + lq for lq in range(3) for c in range(N_CORES)]
                           + [c * 5 + lq for lq in (3, 4) for c in range(N_CORES)])
                s2f0 = None
                for ki, k in enumerate(k_order):
                    c, lq = divmod(k, 5)
                    if "L2G" not in skip or ki == 0:
                        s2f = s2fp.tile([128, 2, F_OUT], F8, tag="s2f")
                        src_ap = (cc_outA[c * 3 + lq] if lq < 3
                                  else cc_outB[c * 2 + lq - 3])
                        (nc.sync if ki % 2 else nc.scalar).dma_start(s2f[:], src_ap)
                        s2f0 = s2f
                    else:
                        s2f = s2f0
                    for f in range(2):
                        lhsT = s2f[:, :, f * 128:(f + 1) * 128]
                        for j, (n0, nw) in enumerate(N_CHUNKS):
                            nc.tensor.matmul(
                                ps2[f][j][:, :nw],
                                lhsT,
                                btr[k][:, :, n0:n0 + nw],
                                start=(ki == 0),
                                stop=(ki == KC - 1),
                                perf_mode=DR,
                            )
                # epilogue: outT = dinv_row * uT + (P2^T + b2)  (fp32)
                for f in range(2):
                    for j, (n0, nw) in enumerate(N_CHUNKS):
                        tmp = tmpp.tile([128, 512], F32, tag="tmp")
                        nc.vector.tensor_mul(
                            tmp[:, :nw], ps2[f][j][:, :nw], dinv_row[:, n0:n0 + nw]
                        )
                        outf = tmpp.tile([128, 512], F32, tag="outf")
                        nc.vector.tensor_add(
                            outf[:, :nw], tmp[:, :nw], p2d[f][j][:, :nw]
                        )
                        nc.sync.dma_start(
                            out_d[f * 128:(f + 1) * 128, n0:n0 + nw], outf[:, :nw]
                        )

    nc.compile()
    return nc


def host_prep(x, edge_index, perturb_first, perturb_last, W1, b1, W2, b2):
    """Index/structure preprocessing + sharding + down-casts of the dense
    input streams. Returns (in_maps, fp8 dtype)."""
    x32 = np.asarray(x, dtype=np.float32)
    x8 = x32.astype(F8_NP)
    # padded global order: node n -> row (n // R) * PB + n % R, zeros in pads
    gidx = (np.arange(N) // R) * PB + np.arange(N) % R
    x8_pad = np.zeros((NPAD, F_IN), dtype=F8_NP)
    x8_pad[gidx] = x8
    # DoubleRow chunk layout [k, p, t, f]: padded row k*256 + t*128 + p
    x8_dr = np.ascontiguousarray(
        x8_pad.reshape(KC, 2, 128, F_IN).transpose(0, 2, 1, 3))
    ei = np.asarray(edge_index)
    src = ei[0].astype(np.int64)
    dst = ei[1].astype(np.int64)
    W1t = np.ascontiguousarray(np.asarray(W1, dtype=np.float32).T).astype(np.float16)
    W2h = np.asarray(W2, dtype=np.float32).astype(np.float16)
    b1 = np.ascontiguousarray(np.asarray(b1, dtype=np.float32))
    b2 = np.ascontiguousarray(np.asarray(b2, dtype=np.float32))
    p1 = np.asarray(perturb_first, dtype=np.float32).astype(np.float16)
    p2 = np.asarray(perturb_last, dtype=np.float32).astype(np.float16)

    deg = np.bincount(dst, minlength=N).astype(np.float32) + 1.0
    dinv = (1.0 / np.sqrt(deg)).astype(np.float32)

    core_of = dst // R
    order = np.argsort(core_of, kind="stable")
    src_s, dst_s = src[order], dst[order]
    counts = np.bincount(core_of, minlength=N_CORES)
    offs = np.concatenate([[0], np.cumsum(counts)])

    in_maps = []
    for c in range(N_CORES):
        lo, hi = offs[c], offs[c + 1]
        s_e = src_s[lo:hi]
        d_e = dst_s[lo:hi] - c * R
        btc = np.zeros(N * R, dtype=np.float32)
        np.add.at(btc, s_e * R + d_e, 1.0)
        rows = np.arange(R, dtype=np.int64)
        btc[(rows + c * R) * R + rows] += 1.0  # self loops
        btc = btc.reshape(N, R)
        btc *= dinv[:, None]  # fold Dinv_src into the count matrix
        btc = btc.astype(F8_NP)
        btc_pad = np.zeros((NPAD, R), dtype=F8_NP)
        btc_pad[gidx] = btc
        bt_dr = np.ascontiguousarray(
            btc_pad.reshape(KC, 2, 128, R).transpose(0, 2, 1, 3))

        rows_sl = slice(c * R, (c + 1) * R)
        in_maps.append({
            "x8": x8_dr,
            "bt": bt_dr,
            "p1t": np.ascontiguousarray(p1[rows_sl].T),
            "p2t": np.ascontiguousarray(p2[rows_sl].T),
            "dinvloc": np.ascontiguousarray(dinv[rows_sl]),
            "w1t": W1t,
            "w2": W2h,
            "b1": b1,
            "b2": b2,
        })
    return in_maps, F8_NP


_NC_CACHE = {}


def kernel(x, edge_index, perturb_first, perturb_last, W1, b1, W2, b2):
    in_maps, _ = host_prep(
        x, edge_index, perturb_first, perturb_last, W1, b1, W2, b2
    )
    key = ("main", 1)
    if key not in _NC_CACHE:
        _NC_CACHE[key] = build_nc(repeat=1)
    nc = _NC_CACHE[key]
    res = run_bass_kernel_spmd(nc, in_maps, list(range(N_CORES)))
    shards = [np.asarray(res.results[c]["outT"]).T for c in range(N_CORES)]
    return np.ascontiguousarray(np.concatenate(shards, axis=0), dtype=np.float32)


# revision 15
# speedup vs baseline: 2.1314x; 2.1314x over previous
"""Trainium2 Bass kernel for a 2-layer GCN encoder (adversarial GCN, N=10000).

Math (per reference):
  conv(X, W, b) = Dinv (A + I) Dinv X W + b,  Dinv = diag(deg^-1/2),
  deg = in-degree(dst) + 1,  A[d, s] = multiplicity of edge (s -> d).
  out = conv2(conv1(x) + perturb_first) + perturb_last

Strategy (8 NeuronCores, 1D node partitioning by dst):
  Let B' = Dinv_src-scaled count matrix: B'[d,s] = (A+I)[d,s] * dinv_s,
  built on host from the edge list + degree histogram (structure data) and
  stored fp8e4m3.  Each core owns 1250 dst rows; its B'^T shard
  [10000, 1250] is loaded ONCE into SBUF as 39 resident [128, 2, 1250]
  k-pair tiles (+ a 16-row tail) and reused by both layers.  The two big
  B-matmuls run in DoubleRow perf mode (fp8 x fp8, 2 contraction rows per
  PE pass) against fp8 features, contracting the narrow 256-col feature
  dim:
    t3 = dinv_d * (B' @ x8)            (layer-1 aggregate, f16 [feat, node])
    s2 = t3^T @ (W1@W2) + (P1+b1) @ W2 (fused W1/W2 stage: W12 is computed
                                        once on device; the perturbation
                                        rides as extra matmul weights, so
                                        s1 is never materialized)
    u  = B' @ fp8(s2);  out = dinv_d * u + P2 + b2
  s2 is exchanged through a single fp8 AllGather (0.31 MB/rank) into one
  shared [10000, 256] buffer; layer 2 gathers 256-row chunks from it with
  two DMAs per chunk.  All intermediates stay in "transposed" layout
  [feat, node] except s2, which is produced naturally row-major for the
  exchange.

Host does index/structure preprocessing (degree histogram, dinv-folded
B'^T shard construction in the DoubleRow tile layout, row-shard slicing /
transposition of perturbs) plus dtype down-casts of the dense input
streams (x -> fp8e4m3, perturbs/weights -> f16); all arithmetic on tensor
data runs on device.
"""

import sys

sys.path.insert(0, "/opt/trn_rl_repo")

import numpy as np
import ml_dtypes

import concourse.bass as bass
import concourse.tile as tile
from concourse import bacc, mybir
from concourse.bass_utils import run_bass_kernel_spmd

N_CORES = 8
N = 10000
R = N // N_CORES  # 1250 rows per core
F_IN = 256
F_HID = 512
F_OUT = 256
PB = 1280  # padded per-core row block (10 full m-tiles)
NPAD = N_CORES * PB  # 10240 = 40 * 256: uniform DoubleRow chunks, no tail
KC = 40  # 256-row DoubleRow contraction chunks

# dst columns per core split into PSUM-bank-sized chunks (<=512 fp32)
N_CHUNKS = [(0, 512), (512, 512), (1024, 226)]
# 1250 = 9*128 + 98 row tiles for the fused W12 (natural-layout) matmul
M_TILES = [(m * 128, min(128, R - m * 128)) for m in range((R + 127) // 128)]
# p1d column-chunk j covering m-tile m0
J_OF_MTILE = [next(j for j, (n0, nw) in enumerate(N_CHUNKS)
                   if n0 <= m0 < n0 + nw) for (m0, _) in M_TILES]

F8 = mybir.dt.float8e4
F8_NP = ml_dtypes.float8_e4m3
F16 = mybir.dt.float16
F32 = mybir.dt.float32
DR = mybir.MatmulPerfMode.DoubleRow
ADD = mybir.AluOpType.add
MUL = mybir.AluOpType.mult


def build_nc(repeat: int = 1, skip: frozenset = frozenset()):
    """skip: subset of {"L1", "D", "AG", "L2"} — timing-attribution
    variants (outputs are garbage when any phase is skipped)."""
    nc = bacc.Bacc("TRN2", target_bir_lowering=False, debug=False, num_devices=N_CORES)

    # ---- DRAM I/O -------------------------------------------------------
    # bt/x pre-laid out on host as [k, p, t, cols]: chunk k is one DMA
    x_d = nc.dram_tensor("x8", [KC, 128, 2, F_IN], F8, kind="ExternalInput")
    bt_d = nc.dram_tensor("bt", [KC, 128, 2, R], F8, kind="ExternalInput")
    p1t_d = nc.dram_tensor("p1t", [F_HID, R], F16, kind="ExternalInput")
    p2t_d = nc.dram_tensor("p2t", [F_OUT, R], F16, kind="ExternalInput")
    dinvloc_d = nc.dram_tensor("dinvloc", [R], F32, kind="ExternalInput")
    w1t_d = nc.dram_tensor("w1t", [F_HID, F_IN], F16, kind="ExternalInput")  # W1^T
    w2_d = nc.dram_tensor("w2", [F_HID, F_OUT], F16, kind="ExternalInput")
    b1_d = nc.dram_tensor("b1", [F_HID], F32, kind="ExternalInput")
    b2_d = nc.dram_tensor("b2", [F_OUT], F32, kind="ExternalInput")
    out_d = nc.dram_tensor("outT", [F_OUT, R], F32, kind="ExternalOutput")

    with tile.TileContext(nc) as tc:
        with (
            tc.tile_pool(name="const", bufs=1) as cpool,
            tc.tile_pool(name="btr", bufs=1) as btrp,
            tc.tile_pool(name="ps", bufs=8, space="PSUM") as ps,
            tc.tile_pool(name="xio", bufs=8) as xio,
            tc.tile_pool(name="s2f", bufs=10) as s2fp,
            tc.tile_pool(name="t3", bufs=1) as t3p,
            tc.tile_pool(name="p1d", bufs=1) as p1dp,
            tc.tile_pool(name="pio", bufs=4) as pio,
            tc.tile_pool(name="tmp", bufs=4) as tmpp,
            tc.tile_pool(name="dram", bufs=1, space="DRAM") as dram,
        ):
            def load_chunk(k, it):
                """One DMA each for the resident bt and x tiles of chunk k."""
                bt = btrp.tile([128, 2, R], F8, name=f"btr{k}_{it}", tag=f"btr{k}")
                (nc.sync if k % 2 else nc.scalar).dma_start(bt[:], bt_d[k])
                xf = None
                if "L1" not in skip:
                    xf = xio.tile([128, 2, F_IN], F8, tag="xio")
                    (nc.scalar if k % 2 else nc.sync).dma_start(xf[:], x_d[k])
                return bt, xf

            # iteration-0 prefetch ahead of the descriptor-heavy constant
            # loads below — the first matmul chain needs only these
            pf0 = {k: load_chunk(k, 0) for k in range(6)}

            # ---- constants ---------------------------------------------
            dinv_row = cpool.tile([128, R], F32)
            nc.sync.dma_start(
                dinv_row[:], dinvloc_d.ap().unsqueeze(0).broadcast_to((128, R))
            )
            b1t = []
            for m in range(4):
                t = cpool.tile([128, 1], F32, name=f"b1t{m}")
                nc.sync.dma_start(t[:], b1_d[m * 128:(m + 1) * 128].unsqueeze(1))
                b1t.append(t)
            b2t = []
            for m in range(2):
                t = cpool.tile([128, 1], F32, name=f"b2t{m}")
                nc.sync.dma_start(t[:], b2_d[m * 128:(m + 1) * 128].unsqueeze(1))
                b2t.append(t)
            w2h = []
            for kk in range(4):
                wh = cpool.tile([128, F_OUT], F16, name=f"w2h{kk}")
                nc.sync.dma_start(wh[:], w2_d[kk * 128:(kk + 1) * 128, :])
                w2h.append(wh)
            w1tt = []
            for kk in range(4):
                wh = cpool.tile([128, F_IN], F16, name=f"w1tt{kk}")
                nc.sync.dma_start(wh[:], w1t_d[kk * 128:(kk + 1) * 128, :])
                w1tt.append(wh)
            # W12 = W1 @ W2 on device, once: [256, 256] f16 as 2 row-tiles
            w12h = []
            for f in range(2):
                psw = ps.tile([128, 512], F32, name=f"psw{f}", tag="ps")
                for kk in range(4):
                    nc.tensor.matmul(
                        psw[:, :F_OUT],
                        w1tt[kk][:, f * 128:(f + 1) * 128],
                        w2h[kk][:],
                        start=(kk == 0),
                        stop=(kk == 3),
                    )
                wh = cpool.tile([128, F_OUT], F16, name=f"w12h{f}")
                nc.vector.tensor_copy(wh[:], psw[:, :F_OUT])
                w12h.append(wh)
            ztc = cpool.tile([128, F_OUT], F8, name="ztc")
            nc.gpsimd.memset(ztc[:], 0.0)

            for it in range(repeat):
                # collective bounce buffers (Shared DRAM output)
                # DR chunk layout [lq, p, t, f]: local row lq*256 + t*128 + p.
                # Split exchange: part A = lq 0..2, part B = lq 3..4 (incl pad);
                # chunk k = rank*5 + lq lives at ccoA[rank*3+lq] / ccoB[rank*2+lq-3]
                cc_in = dram.tile([PB // 256, 128, 2, F_OUT], F8,
                                  name=f"cc_in{it}", tag=f"cci{it}")
                cc_outA = dram.tile([N_CORES * 3, 128, 2, F_OUT], F8,
                                    addr_space="Shared",
                                    name=f"cc_outA{it}", tag=f"ccoA{it}")
                cc_outB = dram.tile([N_CORES * 2, 128, 2, F_OUT], F8,
                                    addr_space="Shared",
                                    name=f"cc_outB{it}", tag=f"ccoB{it}")

                pf = pf0 if it == 0 else {k: load_chunk(k, it) for k in range(3)}
                btr = {}  # resident bt tile per chunk, reused by layer 2

                # ============ Layer 1: t3 = dinv_d * (B' @ x8)^T ============
                t3T = [t3p.tile([128, R], F16, name=f"t3T{f}_{it}", tag=f"t3T{f}")
                       for f in range(2)]
                ps1 = [
                    [ps.tile([128, 512], F32, name=f"ps1_{f}_{j}_{it}", tag="ps")
                     for j in range(3)]
                    for f in range(2)
                ]
                for k in range(KC):
                    bt, xf = pf[k] if k in pf else load_chunk(k, it)
                    btr[k] = bt
                    if "L1" in skip:
                        continue
                    for f in range(2):
                        lhsT = xf[:, :, f * 128:(f + 1) * 128]
                        for j, (n0, nw) in enumerate(N_CHUNKS):
                            nc.tensor.matmul(
                                ps1[f][j][:, :nw],
                                lhsT,
                                bt[:, :, n0:n0 + nw],
                                start=(k == 0),
                                stop=(k == KC - 1),
                                perf_mode=DR,
                            )
                    if k == 12:
                        # p1d[hid][j] = P1^T + b1 (f16) — lands mid-L1-stream
                        # so it is ready for the fused W12 stage right after
                        p1d = [[None] * 3 for _ in range(4)]
                        for hid in range(4):
                            for j, (n0, nw) in enumerate(N_CHUNKS):
                                p1f = pio.tile([128, 512], F16, tag="pio")
                                nc.sync.dma_start(
                                    p1f[:, :nw],
                                    p1t_d[hid * 128:(hid + 1) * 128, n0:n0 + nw],
                                )
                                pd = p1dp.tile([128, 512], F16,
                                               name=f"p1d{hid}_{j}_{it}",
                                               tag=f"p1d{hid}{j}")
                                nc.vector.tensor_scalar_add(
                                    pd[:, :nw], p1f[:, :nw], b1t[hid][:]
                                )
                                p1d[hid][j] = pd
                if "L1" not in skip:
                    for f in range(2):
                        for j, (n0, nw) in enumerate(N_CHUNKS):
                            nc.vector.tensor_mul(
                                t3T[f][:, n0:n0 + nw],
                                ps1[f][j][:, :nw],
                                dinv_row[:, n0:n0 + nw],
                            )
                else:
                    for f in range(2):
                        nc.gpsimd.memset(t3T[f][:], 0.0)
                    p1d = [[None] * 3 for _ in range(4)]
                    for hid in range(4):
                        for j in range(3):
                            pd = p1dp.tile([128, 512], F16,
                                           name=f"p1d{hid}_{j}_{it}",
                                           tag=f"p1d{hid}{j}")
                            nc.gpsimd.memset(pd[:], 0.0)
                            p1d[hid][j] = pd

                # ==== fused W12 stage: s2 = t3^T @ W12 + p1d^T @ W2 (fp8) ====
                if "D" in skip:
                    zt = tmpp.tile([128, F_OUT], F8, tag="s2h")
                    nc.gpsimd.memset(zt[:], 0.0)
                    for (m0, mw) in M_TILES:
                        nc.sync.dma_start(
                            cc_in[m0 // 256, :mw, (m0 // 128) % 2, :], zt[:mw, :])
                    if "AG" not in skip and "AGS" not in skip:
                        nc.gpsimd.collective_compute(
                            "AllGather", mybir.AluOpType.bypass,
                            replica_groups=[list(range(N_CORES))],
                            ins=[cc_in[0:3].opt()], outs=[cc_outA.opt()],
                        )
                else:
                    for mi, (m0, mw) in enumerate(M_TILES):
                        if mi == 6 and "AG" not in skip and "AGS" not in skip:
                            nc.gpsimd.collective_compute(
                                "AllGather", mybir.AluOpType.bypass,
                                replica_groups=[list(range(N_CORES))],
                                ins=[cc_in[0:3].opt()], outs=[cc_outA.opt()],
                            )
                        j = J_OF_MTILE[mi]
                        n0 = N_CHUNKS[j][0]
                        psd = ps.tile([128, 512], F32, name=f"psd_{m0}_{it}",
                                      tag="ps")
                        for kk in range(4):
                            nc.tensor.matmul(
                                psd[:mw, :F_OUT],
                                p1d[kk][j][:, m0 - n0:m0 - n0 + mw],
                                w2h[kk][:],
                                start=(kk == 0),
                                stop=False,
                            )
                        for kk in range(2):
                            nc.tensor.matmul(
                                psd[:mw, :F_OUT],
                                t3T[kk][:, m0:m0 + mw],
                                w12h[kk][:],
                                start=False,
                                stop=(kk == 1),
                            )
                        s2h = tmpp.tile([128, F_OUT], F8, tag="s2h")
                        nc.vector.tensor_copy(s2h[:mw, :], psd[:mw, :F_OUT])
                        nc.sync.dma_start(
                            cc_in[m0 // 256, :mw, (m0 // 128) % 2, :], s2h[:mw, :])
                nc.sync.dma_start(cc_in[4, R - 1152:128, 1, :], ztc[:PB - R, :])

                if "AGS" in skip:
                    # small-payload AllGather probe
                    nc.gpsimd.collective_compute(
                        "AllGather", mybir.AluOpType.bypass,
                        replica_groups=[list(range(N_CORES))],
                        ins=[cc_in[0:1].opt()], outs=[cc_outA[0:8].opt()],
                    )
                elif "AG" not in skip:
                    nc.gpsimd.collective_compute(
                        "AllGather", mybir.AluOpType.bypass,
                        replica_groups=[list(range(N_CORES))],
                        ins=[cc_in[3:5].opt()], outs=[cc_outB.opt()],
                    )
                else:
                    nc.sync.dma_start(cc_outA[0:3], cc_in[0:3])
                    nc.sync.dma_start(cc_outB[0:2], cc_in[3:5])

                # ============ Layer 2: uT = (B' @ s2_full)^T ================
                if "L2" in skip:
                    continue
                # p2d = P2^T + b2 (f16, overlaps the AllGather)
                p2d = [[None] * 3 for _ in range(2)]
                for f in range(2):
                    for j, (n0, nw) in enumerate(N_CHUNKS):
                        p2f = pio.tile([128, 512], F16, tag="pio")
                        nc.sync.dma_start(
                            p2f[:, :nw], p2t_d[f * 128:(f + 1) * 128, n0:n0 + nw]
                        )
                        pd = tmpp.tile([128, 512], F16, name=f"p2d{f}_{j}_{it}",
                                       tag=f"p2d{f}{j}")
                        nc.vector.tensor_scalar_add(pd[:, :nw], p2f[:, :nw], b2t[f][:])
                        p2d[f][j] = pd
                ps2 = [
                    [ps.tile([128, 512], F32, name=f"ps2_{f}_{j}_{it}", tag="ps")
                     for j in range(3)]
                    for f in range(2)
                ]
                k_order = ([c * 5 + lq for lq in range(3) for c in range(N_CORES)]
                           + [c * 5 + lq for lq in (3, 4) for c in range(N_CORES)])
                s2f0 = None
                for ki, k in enumerate(k_order):
                    c, lq = divmod(k, 5)
                    if "L2G" not in skip or ki == 0:
                        s2f = s2fp.tile([128, 2, F_OUT], F8, tag="s2f")
                        src_ap = (cc_outA[c * 3 + lq] if lq < 3
                                  else cc_outB[c * 2 + lq - 3])
                        (nc.sync if ki % 2 else nc.scalar).dma_start(s2f[:], src_ap)
                        s2f0 = s2f
                    else:
                        s2f = s2f0
                    for f in range(2):
                        lhsT = s2f[:, :, f * 128:(f + 1) * 128]
                        for j, (n0, nw) in enumerate(N_CHUNKS):
                            nc.tensor.matmul(
                                ps2[f][j][:, :nw],
                                lhsT,
                                btr[k][:, :, n0:n0 + nw],
                                start=(ki == 0),
                                stop=(ki == KC - 1),
                                perf_mode=DR,
                            )
                # epilogue: outT = dinv_row * uT + (P2^T + b2)  (fp32)
                for f in range(2):
                    for j, (n0, nw) in enumerate(N_CHUNKS):
                        tmp = tmpp.tile([128, 512], F32, tag="tmp")
                        nc.vector.tensor_mul(
                            tmp[:, :nw], ps2[f][j][:, :nw], dinv_row[:, n0:n0 + nw]
                        )
                        outf = tmpp.tile([128, 512], F32, tag="outf")
                        nc.vector.tensor_add(
                            outf[:, :nw], tmp[:, :nw], p2d[f][j][:, :nw]
                        )
                        nc.sync.dma_start(
                            out_d[f * 128:(f + 1) * 128, n0:n0 + nw], outf[:, :nw]
                        )

    nc.compile()
    return nc


def host_prep(x, edge_index, perturb_first, perturb_last, W1, b1, W2, b2):
    """Index/structure preprocessing + sharding + down-casts of the dense
    input streams. Returns (in_maps, fp8 dtype)."""
    x32 = np.asarray(x, dtype=np.float32)
    x8 = x32.astype(F8_NP)
    # padded global order: node n -> row (n // R) * PB + n % R, zeros in pads
    gidx = (np.arange(N) // R) * PB + np.arange(N) % R
    x8_pad = np.zeros((NPAD, F_IN), dtype=F8_NP)
    x8_pad[gidx] = x8
    # DoubleRow chunk layout [k, p, t, f]: padded row k*256 + t*128 + p
    x8_dr = np.ascontiguousarray(
        x8_pad.reshape(KC, 2, 128, F_IN).transpose(0, 2, 1, 3))
    ei = np.asarray(edge_index)
    src = ei[0].astype(np.int64)
    dst = ei[1].astype(np.int64)
    W1t = np.ascontiguousarray(np.asarray(W1, dtype=np.float32).T).astype(np.float16)
    W2h = np.asarray(W2, dtype=np.float32).astype(np.float16)
    b1 = np.ascontiguousarray(np.asarray(b1, dtype=np.float32))
    b2 = np.ascontiguousarray(np.asarray(b2, dtype=np.float32))
    p1 = np.asarray(perturb_first, dtype=np.float32).astype(np.float16)
    p2 = np.asarray(perturb_last, dtype=np.float32).astype(np.float16)

    deg = np.bincount(dst, minlength=N).astype(np.float32) + 1.0
    dinv = (1.0 / np.sqrt(deg)).astype(np.float32)

    core_of = dst // R
    order = np.argsort(core_of, kind="stable")
    src_s, dst_s = src[order], dst[order]
    counts = np.bincount(core_of, minlength=N_CORES)
    offs = np.concatenate([[0], np.cumsum(counts)])

    in_maps = []
    for c in range(N_CORES):
        lo, hi = offs[c], offs[c + 1]
        s_e = src_s[lo:hi]
        d_e = dst_s[lo:hi] - c * R
        btc = np.zeros(N * R, dtype=np.float32)
        np.add.at(btc, s_e * R + d_e, 1.0)
        rows = np.arange(R, dtype=np.int64)
        btc[(rows + c * R) * R + rows] += 1.0  # self loops
        btc = btc.reshape(N, R)
        btc *= dinv[:, None]  # fold Dinv_src into the count matrix
        btc = btc.astype(F8_NP)
        btc_pad = np.zeros((NPAD, R), dtype=F8_NP)
        btc_pad[gidx] = btc
        bt_dr = np.ascontiguousarray(
            btc_pad.reshape(KC, 2, 128, R).transpose(0, 2, 1, 3))

        rows_sl = slice(c * R, (c + 1) * R)
        in_maps.append({
            "x8": x8_dr,
            "bt": bt_dr,
            "p1t": np.ascontiguousarray(p1[rows_sl].T),
            "p2t": np.ascontiguousarray(p2[rows_sl].T),
            "dinvloc": np.ascontiguousarray(dinv[rows_sl]),
            "w1t": W1t,
            "w2": W2h,
            "b1": b1,
            "b2": b2,
        })
    return in_maps, F8_NP


_NC_CACHE = {}


def kernel(x, edge_index, perturb_first, perturb_last, W1, b1, W2, b2):
    in_maps, _ = host_prep(
        x, edge_index, perturb_first, perturb_last, W1, b1, W2, b2
    )
    key = ("main", 1)
    if key not in _NC_CACHE:
        _NC_CACHE[key] = build_nc(repeat=1)
    nc = _NC_CACHE[key]
    res = run_bass_kernel_spmd(nc, in_maps, list(range(N_CORES)))
    shards = [np.asarray(res.results[c]["outT"]).T for c in range(N_CORES)]
    return np.ascontiguousarray(np.concatenate(shards, axis=0), dtype=np.float32)


# revision 17
# speedup vs baseline: 2.7286x; 1.2802x over previous
"""Trainium2 Bass kernel for a 2-layer GCN encoder (adversarial GCN, N=10000).

Math (per reference):
  conv(X, W, b) = Dinv (A + I) Dinv X W + b,  Dinv = diag(deg^-1/2),
  deg = in-degree(dst) + 1,  A[d, s] = multiplicity of edge (s -> d).
  out = conv2(conv1(x) + perturb_first) + perturb_last

Strategy (8 NeuronCores, 1D node partitioning by dst):
  Let B' = Dinv_src-scaled count matrix B'[d,s] = (A+I)[d,s] * dinv_s,
  built on host from the edge list + degree histogram (structure data),
  stored fp8e4m3 in a padded global row order (1280 rows per core block,
  10240 total = 40 uniform 256-row chunks).  Each core owns 1250 dst rows;
  its B'^T shard is loaded ONCE into SBUF as 40 resident [128, 2, 1250]
  k-pair tiles and reused by both layers.  Both big B-matmuls run in
  DoubleRow perf mode (fp8 x fp8, 2 contraction rows per PE pass),
  contracting the narrow 256-col feature dim:
    t3 = dinv_d * (B' @ x8)            (layer-1 aggregate, f16 [feat, node])
    s2 = t3^T @ (W1@W2) + (P1+b1) @ W2 (fused W1/W2 stage: W12 is computed
                                        once on device; the perturbation
                                        rides as extra matmul weights, so
                                        s1 is never materialized)
    u  = B' @ fp8(s2);  out = dinv_d * u + P2 + b2
  s2 is exchanged in fp8 through an AllGather split in two (3/5 fired
  mid-stage to overlap the tail of the W12 stage, 2/5 after), writing
  shared buffers laid out directly in DoubleRow chunk order so each
  layer-2 chunk gather is a single contiguous 64KB DMA.  DMA traffic is
  spread across both HWDGE engines (SP + Activation) for queue
  parallelism.

Host does index/structure preprocessing (degree histogram, dinv-folded
B'^T shard construction in the DoubleRow tile layout, row-shard slicing /
transposition of perturbs) plus dtype down-casts of the dense input
streams (x -> fp8e4m3, perturbs/weights -> f16); all arithmetic on tensor
data runs on device.
"""

import sys

sys.path.insert(0, "/opt/trn_rl_repo")

import numpy as np
import ml_dtypes

import concourse.bass as bass
import concourse.tile as tile
from concourse import bacc, mybir
from concourse.bass_utils import run_bass_kernel_spmd

N_CORES = 8
N = 10000
R = N // N_CORES  # 1250 rows per core
F_IN = 256
F_HID = 512
F_OUT = 256
PB = 1280  # padded per-core row block (10 full m-tiles)
NPAD = N_CORES * PB  # 10240 = 40 * 256: uniform DoubleRow chunks, no tail
KC = 40  # 256-row DoubleRow contraction chunks

# dst columns per core split into PSUM-bank-sized chunks (<=512 fp32)
N_CHUNKS = [(0, 512), (512, 512), (1024, 226)]
# 1250 = 9*128 + 98 row tiles for the fused W12 (natural-layout) matmul
M_TILES = [(m * 128, min(128, R - m * 128)) for m in range((R + 127) // 128)]
# p1d column-chunk j covering m-tile m0
J_OF_MTILE = [next(j for j, (n0, nw) in enumerate(N_CHUNKS)
                   if n0 <= m0 < n0 + nw) for (m0, _) in M_TILES]

F8 = mybir.dt.float8e4
F8_NP = ml_dtypes.float8_e4m3
F16 = mybir.dt.float16
F32 = mybir.dt.float32
DR = mybir.MatmulPerfMode.DoubleRow
ADD = mybir.AluOpType.add
MUL = mybir.AluOpType.mult


def build_nc(repeat: int = 1, skip: frozenset = frozenset()):
    """skip: subset of {"L1", "D", "AG", "L2"} — timing-attribution
    variants (outputs are garbage when any phase is skipped)."""
    nc = bacc.Bacc("TRN2", target_bir_lowering=False, debug=False, num_devices=N_CORES)

    # ---- DRAM I/O -------------------------------------------------------
    # bt/x pre-laid out on host as [k, p, t, cols]: chunk k is one DMA
    x_d = nc.dram_tensor("x8", [KC, 128, 2, F_IN], F8, kind="ExternalInput")
    bt_d = nc.dram_tensor("bt", [KC, 128, 2, R], F8, kind="ExternalInput")
    p1t_d = nc.dram_tensor("p1t", [F_HID, R], F16, kind="ExternalInput")
    p2t_d = nc.dram_tensor("p2t", [F_OUT, R], F16, kind="ExternalInput")
    dinvloc_d = nc.dram_tensor("dinvloc", [R], F32, kind="ExternalInput")
    w1t_d = nc.dram_tensor("w1t", [F_HID, F_IN], F16, kind="ExternalInput")  # W1^T
    w2_d = nc.dram_tensor("w2", [F_HID, F_OUT], F16, kind="ExternalInput")
    b1_d = nc.dram_tensor("b1", [F_HID], F32, kind="ExternalInput")
    b2_d = nc.dram_tensor("b2", [F_OUT], F32, kind="ExternalInput")
    out_d = nc.dram_tensor("outT", [F_OUT, R], F16, kind="ExternalOutput")

    with tile.TileContext(nc) as tc:
        with (
            tc.tile_pool(name="const", bufs=1) as cpool,
            tc.tile_pool(name="btr", bufs=1) as btrp,
            tc.tile_pool(name="ps", bufs=8, space="PSUM") as ps,
            tc.tile_pool(name="xio", bufs=8) as xio,
            tc.tile_pool(name="s2f", bufs=10) as s2fp,
            tc.tile_pool(name="t3", bufs=1) as t3p,
            tc.tile_pool(name="p1d", bufs=1) as p1dp,
            tc.tile_pool(name="pio", bufs=6) as pio,
            tc.tile_pool(name="tmp", bufs=4) as tmpp,
            tc.tile_pool(name="dram", bufs=1, space="DRAM") as dram,
        ):
            def load_chunk(k, it):
                """One DMA each for the resident bt and x tiles of chunk k."""
                bt = btrp.tile([128, 2, R], F8, name=f"btr{k}_{it}", tag=f"btr{k}")
                (nc.sync if k % 2 else nc.scalar).dma_start(bt[:], bt_d[k])
                xf = None
                if "L1" not in skip:
                    xf = xio.tile([128, 2, F_IN], F8, tag="xio")
                    (nc.scalar if k % 2 else nc.sync).dma_start(xf[:], x_d[k])
                return bt, xf

            # iteration-0 prefetch ahead of the descriptor-heavy constant
            # loads below — the first matmul chain needs only these
            pf0 = {k: load_chunk(k, 0) for k in range(6)}

            # ---- constants ---------------------------------------------
            dinv_row = cpool.tile([128, R], F32)
            nc.sync.dma_start(
                dinv_row[:], dinvloc_d.ap().unsqueeze(0).broadcast_to((128, R))
            )
            b1t = []
            for m in range(4):
                t = cpool.tile([128, 1], F32, name=f"b1t{m}")
                nc.sync.dma_start(t[:], b1_d[m * 128:(m + 1) * 128].unsqueeze(1))
                b1t.append(t)
            b2t = []
            for m in range(2):
                t = cpool.tile([128, 1], F32, name=f"b2t{m}")
                nc.sync.dma_start(t[:], b2_d[m * 128:(m + 1) * 128].unsqueeze(1))
                b2t.append(t)
            w2h = []
            for kk in range(4):
                wh = cpool.tile([128, F_OUT], F16, name=f"w2h{kk}")
                nc.sync.dma_start(wh[:], w2_d[kk * 128:(kk + 1) * 128, :])
                w2h.append(wh)
            w1tt = []
            for kk in range(4):
                wh = cpool.tile([128, F_IN], F16, name=f"w1tt{kk}")
                nc.sync.dma_start(wh[:], w1t_d[kk * 128:(kk + 1) * 128, :])
                w1tt.append(wh)
            # W12 = W1 @ W2 on device, once: [256, 256] f16 as 2 row-tiles
            w12h = []
            for f in range(2):
                psw = ps.tile([128, 512], F32, name=f"psw{f}", tag="ps")
                for kk in range(4):
                    nc.tensor.matmul(
                        psw[:, :F_OUT],
                        w1tt[kk][:, f * 128:(f + 1) * 128],
                        w2h[kk][:],
                        start=(kk == 0),
                        stop=(kk == 3),
                    )
                wh = cpool.tile([128, F_OUT], F16, name=f"w12h{f}")
                nc.vector.tensor_copy(wh[:], psw[:, :F_OUT])
                w12h.append(wh)
            ztc = cpool.tile([128, F_OUT], F8, name="ztc")
            nc.gpsimd.memset(ztc[:], 0.0)

            for it in range(repeat):
                # collective bounce buffers (Shared DRAM output)
                # DR chunk layout [lq, p, t, f]: local row lq*256 + t*128 + p.
                # Split exchange: part A = lq 0..2, part B = lq 3..4 (incl pad);
                # chunk k = rank*5 + lq lives at ccoA[rank*3+lq] / ccoB[rank*2+lq-3]
                cc_in = dram.tile([PB // 256, 128, 2, F_OUT], F8,
                                  name=f"cc_in{it}", tag=f"cci{it}")
                cc_outA = dram.tile([N_CORES * 3, 128, 2, F_OUT], F8,
                                    addr_space="Shared",
                                    name=f"cc_outA{it}", tag=f"ccoA{it}")
                cc_outB = dram.tile([N_CORES * 2, 128, 2, F_OUT], F8,
                                    addr_space="Shared",
                                    name=f"cc_outB{it}", tag=f"ccoB{it}")

                pf = pf0 if it == 0 else {k: load_chunk(k, it) for k in range(3)}
                btr = {}  # resident bt tile per chunk, reused by layer 2

                # ============ Layer 1: t3 = dinv_d * (B' @ x8)^T ============
                t3T = [t3p.tile([128, R], F16, name=f"t3T{f}_{it}", tag=f"t3T{f}")
                       for f in range(2)]
                ps1 = [
                    [ps.tile([128, 512], F32, name=f"ps1_{f}_{j}_{it}", tag="ps")
                     for j in range(3)]
                    for f in range(2)
                ]
                for k in range(KC):
                    bt, xf = pf[k] if k in pf else load_chunk(k, it)
                    btr[k] = bt
                    if "L1" in skip:
                        continue
                    for f in range(2):
                        lhsT = xf[:, :, f * 128:(f + 1) * 128]
                        for j, (n0, nw) in enumerate(N_CHUNKS):
                            nc.tensor.matmul(
                                ps1[f][j][:, :nw],
                                lhsT,
                                bt[:, :, n0:n0 + nw],
                                start=(k == 0),
                                stop=(k == KC - 1),
                                perf_mode=DR,
                            )
                    if k == 12:
                        # p1d[hid][j] = P1^T + b1 (f16) — lands mid-L1-stream
                        # so it is ready for the fused W12 stage right after
                        p1d = [[None] * 3 for _ in range(4)]
                        for hid in range(4):
                            for j, (n0, nw) in enumerate(N_CHUNKS):
                                p1f = pio.tile([128, 512], F16, tag="pio")
                                nc.sync.dma_start(
                                    p1f[:, :nw],
                                    p1t_d[hid * 128:(hid + 1) * 128, n0:n0 + nw],
                                )
                                pd = p1dp.tile([128, 512], F16,
                                               name=f"p1d{hid}_{j}_{it}",
                                               tag=f"p1d{hid}{j}")
                                nc.vector.tensor_scalar_add(
                                    pd[:, :nw], p1f[:, :nw], b1t[hid][:]
                                )
                                p1d[hid][j] = pd
                if "L1" not in skip:
                    for f in range(2):
                        for j, (n0, nw) in enumerate(N_CHUNKS):
                            nc.vector.tensor_mul(
                                t3T[f][:, n0:n0 + nw],
                                ps1[f][j][:, :nw],
                                dinv_row[:, n0:n0 + nw],
                            )
                else:
                    for f in range(2):
                        nc.gpsimd.memset(t3T[f][:], 0.0)
                    p1d = [[None] * 3 for _ in range(4)]
                    for hid in range(4):
                        for j in range(3):
                            pd = p1dp.tile([128, 512], F16,
                                           name=f"p1d{hid}_{j}_{it}",
                                           tag=f"p1d{hid}{j}")
                            nc.gpsimd.memset(pd[:], 0.0)
                            p1d[hid][j] = pd

                # ==== fused W12 stage: s2 = t3^T @ W12 + p1d^T @ W2 (fp8) ====
                if "D" in skip:
                    zt = tmpp.tile([128, F_OUT], F8, tag="s2h")
                    nc.gpsimd.memset(zt[:], 0.0)
                    for (m0, mw) in M_TILES:
                        nc.sync.dma_start(
                            cc_in[m0 // 256, :mw, (m0 // 128) % 2, :], zt[:mw, :])
                    if "AG" not in skip and "AGS" not in skip:
                        nc.gpsimd.collective_compute(
                            "AllGather", mybir.AluOpType.bypass,
                            replica_groups=[list(range(N_CORES))],
                            ins=[cc_in[0:3].opt()], outs=[cc_outA.opt()],
                        )
                else:
                    for mi, (m0, mw) in enumerate(M_TILES):
                        if mi == 6 and "AG" not in skip and "AGS" not in skip:
                            nc.gpsimd.collective_compute(
                                "AllGather", mybir.AluOpType.bypass,
                                replica_groups=[list(range(N_CORES))],
                                ins=[cc_in[0:3].opt()], outs=[cc_outA.opt()],
                            )
                        j = J_OF_MTILE[mi]
                        n0 = N_CHUNKS[j][0]
                        psd = ps.tile([128, 512], F32, name=f"psd_{m0}_{it}",
                                      tag="ps")
                        for kk in range(4):
                            nc.tensor.matmul(
                                psd[:mw, :F_OUT],
                                p1d[kk][j][:, m0 - n0:m0 - n0 + mw],
                                w2h[kk][:],
                                start=(kk == 0),
                                stop=False,
                            )
                        for kk in range(2):
                            nc.tensor.matmul(
                                psd[:mw, :F_OUT],
                                t3T[kk][:, m0:m0 + mw],
                                w12h[kk][:],
                                start=False,
                                stop=(kk == 1),
                            )
                        s2h = tmpp.tile([128, F_OUT], F8, tag="s2h")
                        nc.vector.tensor_copy(s2h[:mw, :], psd[:mw, :F_OUT])
                        nc.sync.dma_start(
                            cc_in[m0 // 256, :mw, (m0 // 128) % 2, :], s2h[:mw, :])
                nc.sync.dma_start(cc_in[4, R - 1152:128, 1, :], ztc[:PB - R, :])

                if "AGS" in skip:
                    # small-payload AllGather probe
                    nc.gpsimd.collective_compute(
                        "AllGather", mybir.AluOpType.bypass,
                        replica_groups=[list(range(N_CORES))],
                        ins=[cc_in[0:1].opt()], outs=[cc_outA[0:8].opt()],
                    )
                elif "AG" not in skip:
                    nc.gpsimd.collective_compute(
                        "AllGather", mybir.AluOpType.bypass,
                        replica_groups=[list(range(N_CORES))],
                        ins=[cc_in[3:5].opt()], outs=[cc_outB.opt()],
                    )
                else:
                    nc.sync.dma_start(cc_outA[0:3], cc_in[0:3])
                    nc.sync.dma_start(cc_outB[0:2], cc_in[3:5])

                # ============ Layer 2: uT = (B' @ s2_full)^T ================
                if "L2" in skip:
                    continue
                # p2d = P2^T + b2 (f16, overlaps the AllGather)
                p2d = [[None] * 3 for _ in range(2)]
                for f in range(2):
                    for j, (n0, nw) in enumerate(N_CHUNKS):
                        p2f = pio.tile([128, 512], F16, tag="pio")
                        nc.sync.dma_start(
                            p2f[:, :nw], p2t_d[f * 128:(f + 1) * 128, n0:n0 + nw]
                        )
                        pd = tmpp.tile([128, 512], F16, name=f"p2d{f}_{j}_{it}",
                                       tag=f"p2d{f}{j}")
                        nc.vector.tensor_scalar_add(pd[:, :nw], p2f[:, :nw], b2t[f][:])
                        p2d[f][j] = pd
                ps2 = [
                    [ps.tile([128, 512], F32, name=f"ps2_{f}_{j}_{it}", tag="ps")
                     for j in range(3)]
                    for f in range(2)
                ]
                k_order = ([c * 5 + lq for lq in range(3) for c in range(N_CORES)]
                           + [c * 5 + lq for lq in (3, 4) for c in range(N_CORES)])
                s2f0 = None
                for ki, k in enumerate(k_order):
                    c, lq = divmod(k, 5)
                    if "L2G" not in skip or ki == 0:
                        s2f = s2fp.tile([128, 2, F_OUT], F8, tag="s2f")
                        src_ap = (cc_outA[c * 3 + lq] if lq < 3
                                  else cc_outB[c * 2 + lq - 3])
                        (nc.sync if ki % 2 else nc.scalar).dma_start(s2f[:], src_ap)
                        s2f0 = s2f
                    else:
                        s2f = s2f0
                    for f in range(2):
                        lhsT = s2f[:, :, f * 128:(f + 1) * 128]
                        for j, (n0, nw) in enumerate(N_CHUNKS):
                            nc.tensor.matmul(
                                ps2[f][j][:, :nw],
                                lhsT,
                                btr[k][:, :, n0:n0 + nw],
                                start=(ki == 0),
                                stop=(ki == KC - 1),
                                perf_mode=DR,
                            )
                # epilogue: outT = dinv_row * uT + (P2^T + b2)  (fp32)
                for f in range(2):
                    for j, (n0, nw) in enumerate(N_CHUNKS):
                        tmp = tmpp.tile([128, 512], F32, tag="tmp")
                        nc.vector.tensor_mul(
                            tmp[:, :nw], ps2[f][j][:, :nw], dinv_row[:, n0:n0 + nw]
                        )
                        outf = tmpp.tile([128, 512], F16, tag="outf")
                        nc.vector.tensor_add(
                            outf[:, :nw], tmp[:, :nw], p2d[f][j][:, :nw]
                        )
                        nc.sync.dma_start(
                            out_d[f * 128:(f + 1) * 128, n0:n0 + nw], outf[:, :nw]
                        )

    nc.compile()
    return nc


def host_prep(x, edge_index, perturb_first, perturb_last, W1, b1, W2, b2):
    """Index/structure preprocessing + sharding + down-casts of the dense
    input streams. Returns (in_maps, fp8 dtype)."""
    x32 = np.asarray(x, dtype=np.float32)
    x8 = x32.astype(F8_NP)
    # padded global order: node n -> row (n // R) * PB + n % R, zeros in pads
    gidx = (np.arange(N) // R) * PB + np.arange(N) % R
    x8_pad = np.zeros((NPAD, F_IN), dtype=F8_NP)
    x8_pad[gidx] = x8
    # DoubleRow chunk layout [k, p, t, f]: padded row k*256 + t*128 + p
    x8_dr = np.ascontiguousarray(
        x8_pad.reshape(KC, 2, 128, F_IN).transpose(0, 2, 1, 3))
    ei = np.asarray(edge_index)
    src = ei[0].astype(np.int64)
    dst = ei[1].astype(np.int64)
    W1t = np.ascontiguousarray(np.asarray(W1, dtype=np.float32).T).astype(np.float16)
    W2h = np.asarray(W2, dtype=np.float32).astype(np.float16)
    b1 = np.ascontiguousarray(np.asarray(b1, dtype=np.float32))
    b2 = np.ascontiguousarray(np.asarray(b2, dtype=np.float32))
    p1 = np.asarray(perturb_first, dtype=np.float32).astype(np.float16)
    p2 = np.asarray(perturb_last, dtype=np.float32).astype(np.float16)

    deg = np.bincount(dst, minlength=N).astype(np.float32) + 1.0
    dinv = (1.0 / np.sqrt(deg)).astype(np.float32)

    core_of = dst // R
    order = np.argsort(core_of, kind="stable")
    src_s, dst_s = src[order], dst[order]
    counts = np.bincount(core_of, minlength=N_CORES)
    offs = np.concatenate([[0], np.cumsum(counts)])

    in_maps = []
    for c in range(N_CORES):
        lo, hi = offs[c], offs[c + 1]
        s_e = src_s[lo:hi]
        d_e = dst_s[lo:hi] - c * R
        btc = np.zeros(N * R, dtype=np.float32)
        np.add.at(btc, s_e * R + d_e, 1.0)
        rows = np.arange(R, dtype=np.int64)
        btc[(rows + c * R) * R + rows] += 1.0  # self loops
        btc = btc.reshape(N, R)
        btc *= dinv[:, None]  # fold Dinv_src into the count matrix
        btc = btc.astype(F8_NP)
        btc_pad = np.zeros((NPAD, R), dtype=F8_NP)
        btc_pad[gidx] = btc
        bt_dr = np.ascontiguousarray(
            btc_pad.reshape(KC, 2, 128, R).transpose(0, 2, 1, 3))

        rows_sl = slice(c * R, (c + 1) * R)
        in_maps.append({
            "x8": x8_dr,
            "bt": bt_dr,
            "p1t": np.ascontiguousarray(p1[rows_sl].T),
            "p2t": np.ascontiguousarray(p2[rows_sl].T),
            "dinvloc": np.ascontiguousarray(dinv[rows_sl]),
            "w1t": W1t,
            "w2": W2h,
            "b1": b1,
            "b2": b2,
        })
    return in_maps, F8_NP


_NC_CACHE = {}


def kernel(x, edge_index, perturb_first, perturb_last, W1, b1, W2, b2):
    in_maps, _ = host_prep(
        x, edge_index, perturb_first, perturb_last, W1, b1, W2, b2
    )
    key = ("main", 1)
    if key not in _NC_CACHE:
        _NC_CACHE[key] = build_nc(repeat=1)
    nc = _NC_CACHE[key]
    res = run_bass_kernel_spmd(nc, in_maps, list(range(N_CORES)))
    shards = [np.asarray(res.results[c]["outT"]).T for c in range(N_CORES)]
    return np.ascontiguousarray(np.concatenate(shards, axis=0), dtype=np.float32)
